# revision 6
# baseline (speedup 1.0000x reference)
"""Trainium2 Bass kernel for a BERT layer with relative-position attention bias.

Contract: kernel(**inputs) takes the FULL inputs (as produced by the problem's
setup_inputs) and returns the FULL output [8, 512, 768] float32.

Strategy: data-parallel over batch (B=8 -> one batch element per NeuronCore),
weights replicated, no collectives. Per-core dataflow:

  - activations kept feature-major ([H, S]) for Q/K and the FFN intermediate,
    token-major ([S, H]) for V / attn-out / layernorms.
  - scores computed k-major (scoresT[k, q]) so softmax-normalization can be
    applied per-head via a partition-broadcast multiply and the context matmul
    consumes exp(scores) directly (no probs transpose).
  - relative-position bias via the Toeplitz/shift trick: per (head, q-block)
    A = Q_blk^T @ RT window [128, 640] -> DRAM (bf16) -> shifted strided DMA
    read back as B[q, k] [128, 512] -> transposed-accumulated into the scores
    PSUM with identity matmuls.
  - softmax without max-subtraction (scores are O(1); exact same math).
  - most matmuls in float32r (fp32 storage, ~1e-4 matmul rel err, bf16-speed).
"""
import os
import sys

for _p in ("/opt/trn_rl_repo", os.path.expanduser("~/.axon_site/_ro/trn_rl_repo")):
    if os.path.isdir(_p) and _p not in sys.path:
        sys.path.insert(0, _p)

import numpy as np
import ml_dtypes

import concourse.bass as bass
import concourse.mybir as mybir
import concourse.tile as tile
from concourse import bacc
from concourse.bass_utils import run_bass_kernel_spmd

P = 128
S = 512
H = 768
NH = 12
HD = 64
FF = 3072
MAXPOS = 512
EPS = 1e-12
HB = H // P       # 6 feature blocks
TB = S // P       # 4 token blocks
FB = FF // P      # 24 ff blocks
NJ = 640          # rel window width per q-block
OFF = 127         # shift-read column offset

F32 = mybir.dt.float32
F32R = mybir.dt.float32r
BF16 = mybir.dt.bfloat16

AFT = mybir.ActivationFunctionType
ALU = mybir.AluOpType


def build(trivial_ln1: bool, trivial_ln2: bool):
    nc = bacc.Bacc("TRN2", target_bir_lowering=False, debug=False)

    # ---------------- DRAM I/O ----------------
    d_xT = nc.dram_tensor("xT", [H, S], F32R, kind="ExternalInput")
    d_x = nc.dram_tensor("x_res", [S, H], F32, kind="ExternalInput")
    d_wq = nc.dram_tensor("wq", [H, H], F32R, kind="ExternalInput")
    d_wk = nc.dram_tensor("wk", [H, H], F32R, kind="ExternalInput")
    d_wv = nc.dram_tensor("wv", [H, H], F32R, kind="ExternalInput")
    d_wo = nc.dram_tensor("wo", [H, H], F32R, kind="ExternalInput")
    d_w1 = nc.dram_tensor("w1", [H, FF], F32R, kind="ExternalInput")
    d_w2 = nc.dram_tensor("w2", [FF, H], F32R, kind="ExternalInput")
    d_rt = nc.dram_tensor("rt", [HD, 1024], F32R, kind="ExternalInput")
    d_bq8 = nc.dram_tensor("bq8", [H], F32, kind="ExternalInput")
    d_bk = nc.dram_tensor("bk", [H], F32, kind="ExternalInput")
    d_b1 = nc.dram_tensor("b1f", [FF], F32, kind="ExternalInput")
    d_bo = nc.dram_tensor("bo_row", [1, H], F32R, kind="ExternalInput")
    d_b2 = nc.dram_tensor("b2_row", [1, H], F32R, kind="ExternalInput")
    d_onesc = nc.dram_tensor("ones_col", [P, 1], F32R, kind="ExternalInput")
    d_onesr = nc.dram_tensor("ones_row", [1, P], F32R, kind="ExternalInput")
    d_idb = nc.dram_tensor("ident_bf", [P, P], BF16, kind="ExternalInput")
    d_idf = nc.dram_tensor("ident_f32", [P, P], F32, kind="ExternalInput")
    if not trivial_ln1:
        d_l1s = nc.dram_tensor("ln1s_b", [P, H], F32, kind="ExternalInput")
        d_l1b = nc.dram_tensor("ln1b_b", [P, H], F32, kind="ExternalInput")
    if not trivial_ln2:
        d_l2s = nc.dram_tensor("ln2s_b", [P, H], F32, kind="ExternalInput")
        d_l2b = nc.dram_tensor("ln2b_b", [P, H], F32, kind="ExternalInput")
    d_out = nc.dram_tensor("out", [S, H], F32, kind="ExternalOutput")

    with tile.TileContext(nc) as tc:
        with (
            tc.tile_pool(name="const", bufs=1) as const,
            tc.tile_pool(name="persist", bufs=1) as persist,
            tc.tile_pool(name="wl", bufs=4) as wl_pool,
            tc.tile_pool(name="wr", bufs=4) as wr_pool,
            tc.tile_pool(name="psm", bufs=4, space="PSUM") as psm,
            tc.tile_pool(name="psh", bufs=4, space="PSUM") as psh,
            tc.tile_pool(name="stat", bufs=4) as statp,
            tc.tile_pool(name="evict", bufs=2) as evp,
        ):
            # ---- constants / small tensors ----
            # rel table replicated into both partition halves so that both
            # even heads (Q at partitions 0:64) and odd heads (64:128) can
            # matmul against it (matmul requires equal base partitions).
            rt_sb = const.tile([P, 1024], F32R, name="rt_sb")
            nc.sync.dma_start(rt_sb[0:HD, :], d_rt.ap())
            nc.sync.dma_start(rt_sb[HD : 2 * HD, :], d_rt.ap())
            bq8_sb = const.tile([P, HB], F32, name="bq8_sb")
            nc.sync.dma_start(bq8_sb, d_bq8.ap().rearrange("(o p) -> p o", p=P))
            bk_sb = const.tile([P, HB], F32, name="bk_sb")
            nc.sync.dma_start(bk_sb, d_bk.ap().rearrange("(o p) -> p o", p=P))
            b1_sb = const.tile([P, FB], F32, name="b1_sb")
            nc.sync.dma_start(b1_sb, d_b1.ap().rearrange("(o p) -> p o", p=P))
            bo_sb = const.tile([1, H], F32R, name="bo_sb")
            nc.sync.dma_start(bo_sb, d_bo.ap())
            b2_sb = const.tile([1, H], F32R, name="b2_sb")
            nc.sync.dma_start(b2_sb, d_b2.ap())
            onesc_sb = const.tile([P, 1], F32R, name="onesc_sb")
            nc.sync.dma_start(onesc_sb, d_onesc.ap())
            onesr_sb = const.tile([1, P], F32R, name="onesr_sb")
            nc.sync.dma_start(onesr_sb, d_onesr.ap())
            idb_sb = const.tile([P, P], BF16, name="idb_sb")
            nc.sync.dma_start(idb_sb, d_idb.ap())
            idf_sb = const.tile([P, P], F32, name="idf_sb")
            nc.sync.dma_start(idf_sb, d_idf.ap())
            eps_sb = const.tile([P, 1], F32, name="eps_sb")
            nc.gpsimd.memset(eps_sb, EPS)
            if not trivial_ln1:
                l1s_sb = const.tile([P, H], F32, name="l1s_sb")
                nc.sync.dma_start(l1s_sb, d_l1s.ap())
                l1b_sb = const.tile([P, H], F32, name="l1b_sb")
                nc.sync.dma_start(l1b_sb, d_l1b.ap())
            if not trivial_ln2:
                l2s_sb = const.tile([P, H], F32, name="l2s_sb")
                nc.sync.dma_start(l2s_sb, d_l2s.ap())
                l2b_sb = const.tile([P, H], F32, name="l2b_sb")
                nc.sync.dma_start(l2b_sb, d_l2b.ap())

            # ---- persistent activations ----
            xT_sb = persist.tile([P, HB, S], F32R, name="xT_sb")
            nc.sync.dma_start(xT_sb, d_xT.ap().rearrange("(ko p) s -> p ko s", p=P))
            h1_sb = persist.tile([P, TB, H], F32, name="h1_sb")
            h1T_sb = persist.tile([P, HB, S], F32R, name="h1T_sb")
            if not trivial_ln1:
                h1n_sb = persist.tile([P, TB, H], F32, name="h1n_sb")

            # ================= attention scope =================
            with (
                tc.tile_pool(name="attn", bufs=1) as ap_,
                tc.tile_pool(name="expool", bufs=2) as expool,
                tc.tile_pool(name="Apool", bufs=4) as Apool,
                tc.tile_pool(name="Bpool", bufs=8) as Bpool,
                tc.tile_pool(name="smallp", bufs=3) as smallp,
                tc.tile_pool(name="scr", bufs=8, space="DRAM") as scrp,
            ):
                x_sb = ap_.tile([P, TB, H], F32, name="x_sb")
                nc.sync.dma_start(
                    x_sb, d_x.ap().rearrange("(tb p) h -> p tb h", p=P)
                )
                QT_sb = ap_.tile([P, HB, S], F32R, name="QT_sb")
                KT_sb = ap_.tile([P, HB, S], F32R, name="KT_sb")
                V_sb = ap_.tile([P, TB, H], F32R, name="V_sb")
                ctxT_sb = ap_.tile([P, HB, S], F32R, name="ctxT_sb")

                # ---- QKV projections ----
                for hb in range(HB):
                    wqt = wl_pool.tile([P, HB, P], F32R, tag="wl", name=f"wq_{hb}")
                    nc.sync.dma_start(
                        wqt,
                        d_wq.ap()[:, hb * P : (hb + 1) * P].rearrange(
                            "(ko p) m -> p ko m", p=P
                        ),
                    )
                    psq = psm.tile([P, S], F32, tag="m", name=f"psq_{hb}")
                    for kb in range(HB):
                        nc.tensor.matmul(
                            psq, wqt[:, kb, :], xT_sb[:, kb, :],
                            start=(kb == 0), stop=(kb == HB - 1),
                        )
                    nc.scalar.activation(
                        QT_sb[:, hb, :], psq, AFT.Identity,
                        bias=bq8_sb[:, hb : hb + 1], scale=0.125,
                    )
                for hb in range(HB):
                    wkt = wl_pool.tile([P, HB, P], F32R, tag="wl", name=f"wk_{hb}")
                    nc.sync.dma_start(
                        wkt,
                        d_wk.ap()[:, hb * P : (hb + 1) * P].rearrange(
                            "(ko p) m -> p ko m", p=P
                        ),
                    )
                    psk = psm.tile([P, S], F32, tag="m", name=f"psk_{hb}")
                    for kb in range(HB):
                        nc.tensor.matmul(
                            psk, wkt[:, kb, :], xT_sb[:, kb, :],
                            start=(kb == 0), stop=(kb == HB - 1),
                        )
                    nc.scalar.activation(
                        KT_sb[:, hb, :], psk, AFT.Identity,
                        bias=bk_sb[:, hb : hb + 1], scale=1.0,
                    )
                wv_sb = []
                for kb in range(HB):
                    wvt = wr_pool.tile([P, H], F32R, tag="wr", name=f"wv_{kb}", bufs=6)
                    nc.sync.dma_start(wvt, d_wv.ap()[kb * P : (kb + 1) * P, :])
                    wv_sb.append(wvt)
                for tb in range(TB):
                    for hf in range(2):
                        psv = psh.tile([P, 384], F32, tag="h", name=f"psv_{tb}_{hf}")
                        for kb in range(HB):
                            nc.tensor.matmul(
                                psv,
                                xT_sb[:, kb, tb * P : (tb + 1) * P],
                                wv_sb[kb][:, hf * 384 : (hf + 1) * 384],
                                start=(kb == 0), stop=(kb == HB - 1),
                            )
                        nc.vector.tensor_copy(
                            V_sb[:, tb, hf * 384 : (hf + 1) * 384], psv
                        )

                # ---- attention heads (bias pipelined one head ahead) ----
                def q_head(h):
                    return QT_sb[
                        64 * (h % 2) : 64 * (h % 2) + 64, h // 2, :
                    ]

                def k_head(h):
                    return KT_sb[
                        64 * (h % 2) : 64 * (h % 2) + 64, h // 2, :
                    ]

                B_tiles = {}

                def emit_bias(h):
                    Qh = q_head(h)
                    b0 = 64 * (h % 2)
                    rth = rt_sb[b0 : b0 + HD, :]
                    for qb in range(TB):
                        q0 = qb * P
                        j0 = 384 - q0
                        pb1 = psh.tile([P, 384], F32, tag="h", name=f"pb1_{h}_{qb}")
                        nc.tensor.matmul(
                            pb1, Qh[:, q0 : q0 + P], rth[:, j0 : j0 + 384],
                            start=True, stop=True,
                        )
                        pb2 = psh.tile([P, 384], F32, tag="h", name=f"pb2_{h}_{qb}")
                        nc.tensor.matmul(
                            pb2[:, 0:256], Qh[:, q0 : q0 + P],
                            rth[:, j0 + 384 : j0 + 640],
                            start=True, stop=True,
                        )
                        A_sb = Apool.tile([P, NJ], BF16, tag="A", name=f"A_{h}_{qb}")
                        if qb % 2 == 0:
                            nc.vector.tensor_copy(A_sb[:, 0:384], pb1)
                            nc.scalar.copy(A_sb[:, 384:640], pb2[:, 0:256])
                        else:
                            nc.scalar.copy(A_sb[:, 0:384], pb1)
                            nc.vector.tensor_copy(A_sb[:, 384:640], pb2[:, 0:256])
                        scr = scrp.tile([P, NJ], BF16, tag="scr", name=f"scr_{h}_{qb}")
                        nc.sync.dma_start(scr, A_sb)
                        Bt = Bpool.tile([P, S], BF16, tag="B", name=f"B_{h}_{qb}")
                        shifted = bass.AP(scr.tensor, OFF, [[NJ - 1, P], [1, S]])
                        nc.sync.dma_start(Bt, shifted)
                        B_tiles[(h, qb)] = Bt

                def emit_attn(h):
                    Qh = q_head(h)
                    Kh = k_head(h)
                    ex = expool.tile([P, TB, S], F32R, tag="ex", name=f"ex_{h}")
                    for kb in range(TB):
                        sc = psm.tile([P, S], F32, tag="m", name=f"sc_{h}_{kb}")
                        nc.tensor.matmul(
                            sc, Kh[:, kb * P : (kb + 1) * P], Qh,
                            start=True, stop=False,
                        )
                        for qb in range(TB):
                            nc.tensor.matmul(
                                sc[:, qb * P : (qb + 1) * P],
                                B_tiles[(h, qb)][:, kb * P : (kb + 1) * P],
                                idb_sb,
                                start=False, stop=(qb == TB - 1),
                                skip_group_check=True,
                            )
                        nc.scalar.activation(ex[:, kb, :], sc, AFT.Exp)
                    den = psm.tile([1, S], F32, tag="m", name=f"den_{h}")
                    for kb in range(TB):
                        nc.tensor.matmul(
                            den, onesc_sb, ex[:, kb, :],
                            start=(kb == 0), stop=(kb == TB - 1),
                        )
                    den_sb = smallp.tile([1, S], F32, tag="den", name=f"den_sb_{h}")
                    nc.scalar.copy(den_sb, den)
                    rcp = smallp.tile([1, S], F32, tag="rcp", name=f"rcp_{h}")
                    nc.vector.reciprocal(rcp, den_sb)
                    dbc = smallp.tile([64, S], F32, tag="dbc", name=f"dbc_{h}")
                    nc.gpsimd.partition_broadcast(dbc, rcp)
                    ctx = psm.tile([P, S], F32, tag="m", name=f"ctx_{h}")
                    for kb in range(TB):
                        nc.tensor.matmul(
                            ctx[0:64, :],
                            V_sb[:, kb, 64 * h : 64 * h + 64],
                            ex[:, kb, :],
                            start=(kb == 0), stop=(kb == TB - 1),
                        )
                    nc.vector.tensor_mul(
                        ctxT_sb[64 * (h % 2) : 64 * (h % 2) + 64, h // 2, :],
                        ctx[0:64, :],
                        dbc,
                    )
                    for qb in range(TB):
                        del B_tiles[(h, qb)]

                emit_bias(0)
                for h in range(NH):
                    if h + 1 < NH:
                        emit_bias(h + 1)
                    emit_attn(h)

                # ---- attention output projection + residual + LN1 ----
                wo_sb = []
                for kb in range(HB):
                    wot = wr_pool.tile([P, H], F32R, tag="wr", name=f"wo_{kb}", bufs=6)
                    nc.sync.dma_start(wot, d_wo.ap()[kb * P : (kb + 1) * P, :])
                    wo_sb.append(wot)
                for tb in range(TB):
                    ao_sb = evp.tile([P, H], F32, tag="ao", name=f"ao_{tb}")
                    for hf in range(2):
                        pao = psh.tile([P, 384], F32, tag="h", name=f"pao_{tb}_{hf}")
                        for kb in range(HB):
                            nc.tensor.matmul(
                                pao,
                                ctxT_sb[:, kb, tb * P : (tb + 1) * P],
                                wo_sb[kb][:, hf * 384 : (hf + 1) * 384],
                                start=(kb == 0), stop=False,
                            )
                        nc.tensor.matmul(
                            pao, onesr_sb, bo_sb[:, hf * 384 : (hf + 1) * 384],
                            start=False, stop=True,
                        )
                        nc.vector.tensor_add(
                            ao_sb[:, hf * 384 : (hf + 1) * 384],
                            pao,
                            x_sb[:, tb, hf * 384 : (hf + 1) * 384],
                        )
                    # LN1 (scale/bias folded into W1/b1; h1 = normalized)
                    st = statp.tile([P, 2, 6], F32, tag="st", name=f"st1_{tb}")
                    nc.vector.bn_stats(st[:, 0, :], ao_sb[:, 0:384])
                    nc.vector.bn_stats(st[:, 1, :], ao_sb[:, 384:768])
                    ag = statp.tile([P, 2], F32, tag="ag", name=f"ag1_{tb}")
                    nc.vector.bn_aggr(ag, st)
                    sq = statp.tile([P, 1], F32, tag="sq", name=f"sq1_{tb}")
                    nc.scalar.activation(sq, ag[:, 1:2], AFT.Sqrt, bias=eps_sb)
                    rstd = statp.tile([P, 1], F32, tag="rstd", name=f"rstd1_{tb}")
                    nc.vector.reciprocal(rstd, sq)
                    if trivial_ln1:
                        nc.vector.tensor_scalar(
                            h1_sb[:, tb, :], ao_sb, ag[:, 0:1], rstd,
                            ALU.subtract, ALU.mult,
                        )
                    else:
                        # h1n = normalized (FFN input; scale folded into W1),
                        # h1 = scale*h1n + bias (residual for the second block)
                        nc.vector.tensor_scalar(
                            h1n_sb[:, tb, :], ao_sb, ag[:, 0:1], rstd,
                            ALU.subtract, ALU.mult,
                        )
                        nc.vector.tensor_mul(
                            h1_sb[:, tb, :], h1n_sb[:, tb, :], l1s_sb
                        )
                        nc.vector.tensor_add(
                            h1_sb[:, tb, :], h1_sb[:, tb, :], l1b_sb
                        )

                # transpose LN1-normalized hidden -> feature-major for FFN
                tsrc = h1_sb if trivial_ln1 else h1n_sb
                for hb in range(HB):
                    pt = psm.tile([P, S], F32, tag="m", name=f"pt_{hb}")
                    for tb in range(TB):
                        nc.tensor.transpose(
                            pt[:, tb * P : (tb + 1) * P],
                            tsrc[:, tb, hb * P : (hb + 1) * P],
                            idf_sb,
                        )
                    if hb % 2 == 0:
                        nc.vector.tensor_copy(h1T_sb[:, hb, :], pt)
                    else:
                        nc.scalar.copy(h1T_sb[:, hb, :], pt)

            # ================= FFN scope =================
            with (
                tc.tile_pool(name="gpool", bufs=FB) as gpool,
                tc.tile_pool(name="ypool", bufs=1) as ypool,
            ):
                y_sb = ypool.tile([P, TB, H], F32, name="y_sb")
                g_tiles = []
                for f in range(FB):
                    w1t = wl_pool.tile([P, HB, P], F32R, tag="wl", name=f"w1_{f}")
                    nc.sync.dma_start(
                        w1t,
                        d_w1.ap()[:, f * P : (f + 1) * P].rearrange(
                            "(ko p) m -> p ko m", p=P
                        ),
                    )
                    pf = psm.tile([P, S], F32, tag="m", name=f"pf_{f}")
                    for kb in range(HB):
                        nc.tensor.matmul(
                            pf, w1t[:, kb, :], h1T_sb[:, kb, :],
                            start=(kb == 0), stop=(kb == HB - 1),
                        )
                    g = gpool.tile([P, S], F32R, tag="g", name=f"g_{f}")
                    nc.scalar.activation(
                        g, pf, AFT.Gelu, bias=b1_sb[:, f : f + 1]
                    )
                    g_tiles.append(g)

                for hf in range(2):
                    py = [
                        psh.tile([P, 384], F32, tag="h", name=f"py_{hf}_{tb}")
                        for tb in range(TB)
                    ]
                    for f in range(FB):
                        w2t = wr_pool.tile(
                            [P, 384], F32R, tag="w2", name=f"w2_{hf}_{f}"
                        )
                        nc.sync.dma_start(
                            w2t,
                            d_w2.ap()[
                                f * P : (f + 1) * P, hf * 384 : (hf + 1) * 384
                            ],
                        )
                        for tb in range(TB):
                            nc.tensor.matmul(
                                py[tb],
                                g_tiles[f][:, tb * P : (tb + 1) * P],
                                w2t,
                                start=(f == 0), stop=False,
                            )
                    for tb in range(TB):
                        nc.tensor.matmul(
                            py[tb], onesr_sb, b2_sb[:, hf * 384 : (hf + 1) * 384],
                            start=False, stop=True,
                        )
                        nc.vector.tensor_add(
                            y_sb[:, tb, hf * 384 : (hf + 1) * 384],
                            py[tb],
                            h1_sb[:, tb, hf * 384 : (hf + 1) * 384],
                        )

                # LN2 -> output
                for tb in range(TB):
                    st = statp.tile([P, 2, 6], F32, tag="st", name=f"st2_{tb}")
                    nc.vector.bn_stats(st[:, 0, :], y_sb[:, tb, 0:384])
                    nc.vector.bn_stats(st[:, 1, :], y_sb[:, tb, 384:768])
                    ag = statp.tile([P, 2], F32, tag="ag", name=f"ag2_{tb}")
                    nc.vector.bn_aggr(ag, st)
                    sq = statp.tile([P, 1], F32, tag="sq", name=f"sq2_{tb}")
                    nc.scalar.activation(sq, ag[:, 1:2], AFT.Sqrt, bias=eps_sb)
                    rstd = statp.tile([P, 1], F32, tag="rstd", name=f"rstd2_{tb}")
                    nc.vector.reciprocal(rstd, sq)
                    o_sb = evp.tile([P, H], F32, tag="o", name=f"o_{tb}")
                    nc.vector.tensor_scalar(
                        o_sb, y_sb[:, tb, :], ag[:, 0:1], rstd,
                        ALU.subtract, ALU.mult,
                    )
                    if not trivial_ln2:
                        nc.vector.tensor_mul(o_sb, o_sb, l2s_sb)
                        nc.vector.tensor_add(o_sb, o_sb, l2b_sb)
                    nc.sync.dma_start(
                        d_out.ap()[tb * P : (tb + 1) * P, :], o_sb
                    )

    nc.compile()
    return nc


_CACHE = {}


def _get_nc(trivial_ln1, trivial_ln2):
    key = (trivial_ln1, trivial_ln2)
    if key not in _CACHE:
        _CACHE[key] = build(trivial_ln1, trivial_ln2)
    return _CACHE[key]


def _prepare(inputs):
    f32 = np.float32
    x = np.asarray(inputs["hidden_states"], f32)            # [B, S, H]
    mask = np.asarray(inputs["attention_mask"])
    assert mask.all(), "kernel assumes an all-true attention mask"
    Wq = np.asarray(inputs["Wq"], f32)
    bq = np.asarray(inputs["bq"], f32)
    Wk = np.asarray(inputs["Wk"], f32)
    bk = np.asarray(inputs["bk"], f32)
    Wv = np.asarray(inputs["Wv"], f32)
    bv = np.asarray(inputs["bv"], f32)
    Wo = np.asarray(inputs["Wo"], f32)
    bo = np.asarray(inputs["bo"], f32)
    rel = np.asarray(inputs["rel_table"], f32)              # [1023, 64]
    l1s = np.asarray(inputs["ln1_scale"], f32)
    l1b = np.asarray(inputs["ln1_bias"], f32)
    W1 = np.asarray(inputs["W1"], f32)
    b1 = np.asarray(inputs["b1"], f32)
    W2 = np.asarray(inputs["W2"], f32)
    b2 = np.asarray(inputs["b2"], f32)
    l2s = np.asarray(inputs["ln2_scale"], f32)
    l2b = np.asarray(inputs["ln2_bias"], f32)

    B = x.shape[0]
    trivial_ln1 = bool(np.all(l1s == 1.0) and np.all(l1b == 0.0))
    trivial_ln2 = bool(np.all(l2s == 1.0) and np.all(l2b == 0.0))

    # host-side folds (exact algebra)
    bo_p = bo + bv @ Wo                      # V-bias folded via softmax row-sum
    RT = np.zeros((HD, 1024), f32)
    RT[:, :1023] = 8.0 * rel[::-1].T         # Q pre-scaled by 1/8; x8 here
    W1f = l1s[:, None] * W1
    b1f = b1 + l1b @ W1

    common = {
        "wq": np.ascontiguousarray(Wq),
        "wk": np.ascontiguousarray(Wk),
        "wv": np.ascontiguousarray(Wv),
        "wo": np.ascontiguousarray(Wo),
        "w1": np.ascontiguousarray(W1f),
        "w2": np.ascontiguousarray(W2),
        "rt": RT,
        "bq8": np.ascontiguousarray(bq / 8.0),
        "bk": np.ascontiguousarray(bk),
        "b1f": np.ascontiguousarray(b1f),
        "bo_row": np.ascontiguousarray(bo_p[None, :]),
        "b2_row": np.ascontiguousarray(b2[None, :]),
        "ones_col": np.ones((P, 1), f32),
        "ones_row": np.ones((1, P), f32),
        "ident_bf": np.eye(P, dtype=ml_dtypes.bfloat16),
        "ident_f32": np.eye(P, dtype=f32),
    }
    if not trivial_ln1:
        common["ln1s_b"] = np.broadcast_to(l1s, (P, H)).copy()
        common["ln1b_b"] = np.broadcast_to(l1b, (P, H)).copy()
    if not trivial_ln2:
        common["ln2s_b"] = np.broadcast_to(l2s, (P, H)).copy()
        common["ln2b_b"] = np.broadcast_to(l2b, (P, H)).copy()

    in_maps = []
    for b in range(B):
        m = dict(common)
        m["xT"] = np.ascontiguousarray(x[b].T)
        m["x_res"] = np.ascontiguousarray(x[b])
        in_maps.append(m)
    return in_maps, trivial_ln1, trivial_ln2, x.dtype


def run(inputs, trace=False, **kw):
    in_maps, t1, t2, dt = _prepare(inputs)
    nc = _get_nc(t1, t2)
    res = run_bass_kernel_spmd(
        nc, in_maps, core_ids=list(range(len(in_maps))), trace=trace, **kw
    )
    out = np.stack([res.results[c]["out"] for c in range(len(in_maps))])
    return out.astype(dt, copy=False), res


def kernel(**inputs) -> np.ndarray:
    out, _ = run(inputs, trace=False)
    return out


# revision 9
# speedup vs baseline: 1.1500x; 1.1500x over previous
"""Trainium2 Bass kernel for a BERT layer with relative-position attention bias.

Contract: kernel(**inputs) takes the FULL inputs (as produced by the problem's
setup_inputs) and returns the FULL output [8, 512, 768] float32.

Strategy: data-parallel over batch (B=8 -> one batch element per NeuronCore),
weights replicated, no collectives. Per-core dataflow:

  - activations kept feature-major ([H, S]) for Q/K and the FFN intermediate,
    token-major ([S, H]) for V / attn-out / layernorms.
  - scores computed k-major (scoresT[k, q]) so softmax normalization is a
    per-head partition-broadcast multiply and the context matmul consumes
    exp(scores) directly (no probs transpose).
  - relative-position bias via the Toeplitz/shift trick: per (head, q-block)
    A = Q_blk^T @ RT window [128, 640] -> DRAM (fp16) -> shifted strided DMA
    read back as B[q, k] [128, 512] -> transposed-accumulated into the scores
    PSUM with identity matmuls.
  - softmax denominator accumulated by the context matmul itself via an
    interleaved ones-column in V (65 columns per head).
  - softmax without max-subtraction (scores are O(1); same math).
  - matmuls in fp16 (fast weight loads, ~3e-4 matmul rel err); fp32
    accumulation in PSUM, fp32 layernorm/residual arithmetic.
"""
import os
import sys

for _p in ("/opt/trn_rl_repo", os.path.expanduser("~/.axon_site/_ro/trn_rl_repo")):
    if os.path.isdir(_p) and _p not in sys.path:
        sys.path.insert(0, _p)

import numpy as np
import ml_dtypes

import concourse.bass as bass
import concourse.mybir as mybir
import concourse.tile as tile
from concourse import bacc
from concourse.bass_utils import run_bass_kernel_spmd

P = 128
S = 512
H = 768
NH = 12
HD = 64
FF = 3072
MAXPOS = 512
EPS = 1e-12
HB = H // P       # 6 feature blocks
TB = S // P       # 4 token blocks
FB = FF // P      # 24 ff blocks
NJ = 640          # rel window width per q-block
OFF = 127         # shift-read column offset
VW = NH * (HD + 1)  # V row width: 12 heads x (64 value cols + 1 ones col)

F32 = mybir.dt.float32
F16 = mybir.dt.float16

AFT = mybir.ActivationFunctionType
ALU = mybir.AluOpType


def build(trivial_ln1: bool, trivial_ln2: bool):
    nc = bacc.Bacc("TRN2", target_bir_lowering=False, debug=False)

    # ---------------- DRAM I/O ----------------
    d_xT = nc.dram_tensor("xT", [H, S], F16, kind="ExternalInput")
    d_x = nc.dram_tensor("x_res", [S, H], F32, kind="ExternalInput")
    d_wq = nc.dram_tensor("wq", [H, H], F16, kind="ExternalInput")
    d_wk = nc.dram_tensor("wk", [H, H], F16, kind="ExternalInput")
    d_wv = nc.dram_tensor("wv", [H, H], F16, kind="ExternalInput")
    d_wo = nc.dram_tensor("wo", [H, H], F16, kind="ExternalInput")
    d_w1 = nc.dram_tensor("w1", [H, FF], F16, kind="ExternalInput")
    d_w2 = nc.dram_tensor("w2", [FF, H], F16, kind="ExternalInput")
    d_rt = nc.dram_tensor("rt", [HD, 1024], F16, kind="ExternalInput")
    d_bq8 = nc.dram_tensor("bq8", [H], F32, kind="ExternalInput")
    d_bk = nc.dram_tensor("bk", [H], F32, kind="ExternalInput")
    d_b1 = nc.dram_tensor("b1f", [FF], F32, kind="ExternalInput")
    d_bo = nc.dram_tensor("bo_row", [1, H], F16, kind="ExternalInput")
    d_b2 = nc.dram_tensor("b2_row", [1, H], F16, kind="ExternalInput")
    d_onesr = nc.dram_tensor("ones_row", [1, P], F16, kind="ExternalInput")
    d_idh = nc.dram_tensor("ident_f16", [P, P], F16, kind="ExternalInput")
    d_idf = nc.dram_tensor("ident_f32", [P, P], F32, kind="ExternalInput")
    if not trivial_ln1:
        d_l1s = nc.dram_tensor("ln1s_b", [P, H], F32, kind="ExternalInput")
        d_l1b = nc.dram_tensor("ln1b_b", [P, H], F32, kind="ExternalInput")
    if not trivial_ln2:
        d_l2s = nc.dram_tensor("ln2s_b", [P, H], F32, kind="ExternalInput")
        d_l2b = nc.dram_tensor("ln2b_b", [P, H], F32, kind="ExternalInput")
    d_out = nc.dram_tensor("out", [S, H], F32, kind="ExternalOutput")

    with tile.TileContext(nc) as tc:
        with (
            tc.tile_pool(name="const", bufs=1) as const,
            tc.tile_pool(name="persist", bufs=1) as persist,
            tc.tile_pool(name="wl", bufs=4) as wl_pool,
            tc.tile_pool(name="wr", bufs=4) as wr_pool,
            tc.tile_pool(name="psm", bufs=4, space="PSUM") as psm,
            tc.tile_pool(name="psh", bufs=4, space="PSUM") as psh,
            tc.tile_pool(name="stat", bufs=4) as statp,
            tc.tile_pool(name="evict", bufs=2) as evp,
        ):
            # ---- constants / small tensors ----
            # rel table replicated into both partition halves so both even
            # heads (Q at partitions 0:64) and odd heads (64:128) can matmul
            # against it (matmul requires equal base partitions).
            rt_sb = const.tile([P, 1024], F16, name="rt_sb")
            nc.sync.dma_start(rt_sb[0:HD, :], d_rt.ap())
            nc.sync.dma_start(rt_sb[HD : 2 * HD, :], d_rt.ap())
            bq8_sb = const.tile([P, HB], F32, name="bq8_sb")
            nc.sync.dma_start(bq8_sb, d_bq8.ap().rearrange("(o p) -> p o", p=P))
            bk_sb = const.tile([P, HB], F32, name="bk_sb")
            nc.sync.dma_start(bk_sb, d_bk.ap().rearrange("(o p) -> p o", p=P))
            b1_sb = const.tile([P, FB], F32, name="b1_sb")
            nc.sync.dma_start(b1_sb, d_b1.ap().rearrange("(o p) -> p o", p=P))
            bo_sb = const.tile([1, H], F16, name="bo_sb")
            nc.sync.dma_start(bo_sb, d_bo.ap())
            b2_sb = const.tile([1, H], F16, name="b2_sb")
            nc.sync.dma_start(b2_sb, d_b2.ap())
            onesr_sb = const.tile([1, P], F16, name="onesr_sb")
            nc.sync.dma_start(onesr_sb, d_onesr.ap())
            idh_sb = const.tile([P, P], F16, name="idh_sb")
            nc.sync.dma_start(idh_sb, d_idh.ap())
            idf_sb = const.tile([P, P], F32, name="idf_sb")
            nc.sync.dma_start(idf_sb, d_idf.ap())
            eps_sb = const.tile([P, 1], F32, name="eps_sb")
            nc.gpsimd.memset(eps_sb, EPS)
            if not trivial_ln1:
                l1s_sb = const.tile([P, H], F32, name="l1s_sb")
                nc.sync.dma_start(l1s_sb, d_l1s.ap())
                l1b_sb = const.tile([P, H], F32, name="l1b_sb")
                nc.sync.dma_start(l1b_sb, d_l1b.ap())
            if not trivial_ln2:
                l2s_sb = const.tile([P, H], F32, name="l2s_sb")
                nc.sync.dma_start(l2s_sb, d_l2s.ap())
                l2b_sb = const.tile([P, H], F32, name="l2b_sb")
                nc.sync.dma_start(l2b_sb, d_l2b.ap())

            # ---- persistent activations ----
            xT_sb = persist.tile([P, HB, S], F16, name="xT_sb")
            nc.sync.dma_start(xT_sb, d_xT.ap().rearrange("(ko p) s -> p ko s", p=P))
            h1_sb = persist.tile([P, TB, H], F32, name="h1_sb")
            h1T_sb = persist.tile([P, HB, S], F16, name="h1T_sb")
            if not trivial_ln1:
                h1n_sb = persist.tile([P, TB, H], F32, name="h1n_sb")

            # ================= attention scope =================
            with (
                tc.tile_pool(name="attn", bufs=1) as ap_,
                tc.tile_pool(name="expool", bufs=2) as expool,
                tc.tile_pool(name="Apool", bufs=4) as Apool,
                tc.tile_pool(name="Bpool", bufs=8) as Bpool,
                tc.tile_pool(name="smallp", bufs=3) as smallp,
                tc.tile_pool(name="scr", bufs=8, space="DRAM") as scrp,
            ):
                x_sb = ap_.tile([P, TB, H], F32, name="x_sb")
                nc.sync.dma_start(
                    x_sb, d_x.ap().rearrange("(tb p) h -> p tb h", p=P)
                )
                QT_sb = ap_.tile([P, HB, S], F16, name="QT_sb")
                KT_sb = ap_.tile([P, HB, S], F16, name="KT_sb")
                # V with an interleaved ones-column per head: head h occupies
                # columns [65h, 65h+64), column 65h+64 is ones so the context
                # matmul also produces the softmax denominator in row 64.
                V_sb = ap_.tile([P, TB, VW], F16, name="V_sb")
                nc.gpsimd.memset(V_sb, 1.0)
                ctxT_sb = ap_.tile([P, HB, S], F16, name="ctxT_sb")

                # ---- QKV projections ----
                for hb in range(HB):
                    wqt = wl_pool.tile([P, HB, P], F16, tag="wl", name=f"wq_{hb}")
                    nc.sync.dma_start(
                        wqt,
                        d_wq.ap()[:, hb * P : (hb + 1) * P].rearrange(
                            "(ko p) m -> p ko m", p=P
                        ),
                    )
                    psq = psm.tile([P, S], F32, tag="m", name=f"psq_{hb}")
                    for kb in range(HB):
                        nc.tensor.matmul(
                            psq, wqt[:, kb, :], xT_sb[:, kb, :],
                            start=(kb == 0), stop=(kb == HB - 1),
                        )
                    nc.scalar.activation(
                        QT_sb[:, hb, :], psq, AFT.Identity,
                        bias=bq8_sb[:, hb : hb + 1], scale=0.125,
                    )
                for hb in range(HB):
                    wkt = wl_pool.tile([P, HB, P], F16, tag="wl", name=f"wk_{hb}")
                    nc.sync.dma_start(
                        wkt,
                        d_wk.ap()[:, hb * P : (hb + 1) * P].rearrange(
                            "(ko p) m -> p ko m", p=P
                        ),
                    )
                    psk = psm.tile([P, S], F32, tag="m", name=f"psk_{hb}")
                    for kb in range(HB):
                        nc.tensor.matmul(
                            psk, wkt[:, kb, :], xT_sb[:, kb, :],
                            start=(kb == 0), stop=(kb == HB - 1),
                        )
                    nc.scalar.activation(
                        KT_sb[:, hb, :], psk, AFT.Identity,
                        bias=bk_sb[:, hb : hb + 1], scale=1.0,
                    )
                wv_sb = []
                for kb in range(HB):
                    wvt = wr_pool.tile([P, H], F16, tag="wr", name=f"wv_{kb}", bufs=6)
                    nc.sync.dma_start(wvt, d_wv.ap()[kb * P : (kb + 1) * P, :])
                    wv_sb.append(wvt)
                for tb in range(TB):
                    for hf in range(2):
                        psv = psh.tile([P, 384], F32, tag="h", name=f"psv_{tb}_{hf}")
                        for kb in range(HB):
                            nc.tensor.matmul(
                                psv,
                                xT_sb[:, kb, tb * P : (tb + 1) * P],
                                wv_sb[kb][:, hf * 384 : (hf + 1) * 384],
                                start=(kb == 0), stop=(kb == HB - 1),
                            )
                        # scatter 6 heads x 64 cols into the 65-strided layout
                        nc.vector.tensor_copy(
                            V_sb[:, tb, :]
                            .rearrange("p (nh c) -> p nh c", nh=NH, c=HD + 1)[
                                :, 6 * hf : 6 * hf + 6, 0:HD
                            ],
                            psv.rearrange("p (nh c) -> p nh c", nh=6, c=HD),
                        )

                # ---- attention heads (bias pipelined one head ahead) ----
                def q_head(h):
                    return QT_sb[64 * (h % 2) : 64 * (h % 2) + 64, h // 2, :]

                def k_head(h):
                    return KT_sb[64 * (h % 2) : 64 * (h % 2) + 64, h // 2, :]

                B_tiles = {}

                def emit_bias(h):
                    Qh = q_head(h)
                    b0 = 64 * (h % 2)
                    rth = rt_sb[b0 : b0 + HD, :]
                    for qb in range(TB):
                        q0 = qb * P
                        j0 = 384 - q0
                        pb1 = psh.tile([P, 384], F32, tag="h", name=f"pb1_{h}_{qb}")
                        nc.tensor.matmul(
                            pb1, Qh[:, q0 : q0 + P], rth[:, j0 : j0 + 384],
                            start=True, stop=True,
                        )
                        pb2 = psh.tile([P, 384], F32, tag="h", name=f"pb2_{h}_{qb}")
                        nc.tensor.matmul(
                            pb2[:, 0:256], Qh[:, q0 : q0 + P],
                            rth[:, j0 + 384 : j0 + 640],
                            start=True, stop=True,
                        )
                        A_sb = Apool.tile([P, NJ], F16, tag="A", name=f"A_{h}_{qb}")
                        if qb % 2 == 0:
                            nc.vector.tensor_copy(A_sb[:, 0:384], pb1)
                            nc.scalar.copy(A_sb[:, 384:640], pb2[:, 0:256])
                        else:
                            nc.scalar.copy(A_sb[:, 0:384], pb1)
                            nc.vector.tensor_copy(A_sb[:, 384:640], pb2[:, 0:256])
                        scr = scrp.tile([P, NJ], F16, tag="scr", name=f"scr_{h}_{qb}")
                        nc.sync.dma_start(scr, A_sb)
                        Bt = Bpool.tile([P, S], F16, tag="B", name=f"B_{h}_{qb}")
                        shifted = bass.AP(scr.tensor, OFF, [[NJ - 1, P], [1, S]])
                        nc.sync.dma_start(Bt, shifted)
                        B_tiles[(h, qb)] = Bt

                def emit_attn(h):
                    Qh = q_head(h)
                    Kh = k_head(h)
                    ex = expool.tile([P, TB, S], F16, tag="ex", name=f"ex_{h}")
                    for kb in range(TB):
                        sc = psm.tile([P, S], F32, tag="m", name=f"sc_{h}_{kb}")
                        nc.tensor.matmul(
                            sc, Kh[:, kb * P : (kb + 1) * P], Qh,
                            start=True, stop=False,
                        )
                        for qb in range(TB):
                            nc.tensor.matmul(
                                sc[:, qb * P : (qb + 1) * P],
                                B_tiles[(h, qb)][:, kb * P : (kb + 1) * P],
                                idh_sb,
                                start=False, stop=(qb == TB - 1),
                                skip_group_check=True,
                            )
                        nc.scalar.activation(ex[:, kb, :], sc, AFT.Exp)
                    # context + denominator (ones col) in one accumulation
                    ctx = psm.tile([P, S], F32, tag="m", name=f"ctx_{h}")
                    for kb in range(TB):
                        nc.tensor.matmul(
                            ctx[0 : HD + 1, :],
                            V_sb[:, kb, 65 * h : 65 * h + HD + 1],
                            ex[:, kb, :],
                            start=(kb == 0), stop=(kb == TB - 1),
                        )
                    den_sb = smallp.tile([1, S], F32, tag="den", name=f"den_sb_{h}")
                    # reciprocal via ACT ln/exp (DVE reciprocal on one
                    # partition costs ~4us; ln+exp is ~1.1us on ACT)
                    nc.scalar.activation(den_sb, ctx[HD : HD + 1, :], AFT.Ln)
                    rcp = smallp.tile([1, S], F32, tag="rcp", name=f"rcp_{h}")
                    nc.scalar.activation(rcp, den_sb, AFT.Exp, scale=-1.0)
                    dbc = smallp.tile([64, S], F32, tag="dbc", name=f"dbc_{h}")
                    nc.gpsimd.partition_broadcast(dbc, rcp)
                    nc.vector.tensor_mul(
                        ctxT_sb[64 * (h % 2) : 64 * (h % 2) + 64, h // 2, :],
                        ctx[0:HD, :],
                        dbc,
                    )
                    for qb in range(TB):
                        del B_tiles[(h, qb)]

                emit_bias(0)
                for h in range(NH):
                    if h + 1 < NH:
                        emit_bias(h + 1)
                    emit_attn(h)

                # ---- attention output projection + residual + LN1 ----
                wo_sb = []
                for kb in range(HB):
                    wot = wr_pool.tile([P, H], F16, tag="wr", name=f"wo_{kb}", bufs=6)
                    nc.sync.dma_start(wot, d_wo.ap()[kb * P : (kb + 1) * P, :])
                    wo_sb.append(wot)
                for tb in range(TB):
                    ao_sb = evp.tile([P, H], F32, tag="ao", name=f"ao_{tb}")
                    for hf in range(2):
                        pao = psh.tile([P, 384], F32, tag="h", name=f"pao_{tb}_{hf}")
                        for kb in range(HB):
                            nc.tensor.matmul(
                                pao,
                                ctxT_sb[:, kb, tb * P : (tb + 1) * P],
                                wo_sb[kb][:, hf * 384 : (hf + 1) * 384],
                                start=(kb == 0), stop=False,
                            )
                        nc.tensor.matmul(
                            pao, onesr_sb, bo_sb[:, hf * 384 : (hf + 1) * 384],
                            start=False, stop=True,
                        )
                        nc.vector.tensor_add(
                            ao_sb[:, hf * 384 : (hf + 1) * 384],
                            pao,
                            x_sb[:, tb, hf * 384 : (hf + 1) * 384],
                        )
                    # LN1 (scale/bias folded into W1/b1; h1 = normalized)
                    st = statp.tile([P, 2, 6], F32, tag="st", name=f"st1_{tb}")
                    nc.vector.bn_stats(st[:, 0, :], ao_sb[:, 0:384])
                    nc.vector.bn_stats(st[:, 1, :], ao_sb[:, 384:768])
                    ag = statp.tile([P, 2], F32, tag="ag", name=f"ag1_{tb}")
                    nc.vector.bn_aggr(ag, st)
                    sq = statp.tile([P, 1], F32, tag="sq", name=f"sq1_{tb}")
                    nc.scalar.activation(sq, ag[:, 1:2], AFT.Sqrt, bias=eps_sb)
                    rstd = statp.tile([P, 1], F32, tag="rstd", name=f"rstd1_{tb}")
                    nc.vector.reciprocal(rstd, sq)
                    if trivial_ln1:
                        nc.vector.tensor_scalar(
                            h1_sb[:, tb, :], ao_sb, ag[:, 0:1], rstd,
                            ALU.subtract, ALU.mult,
                        )
                    else:
                        nc.vector.tensor_scalar(
                            h1n_sb[:, tb, :], ao_sb, ag[:, 0:1], rstd,
                            ALU.subtract, ALU.mult,
                        )
                        nc.vector.tensor_mul(
                            h1_sb[:, tb, :], h1n_sb[:, tb, :], l1s_sb
                        )
                        nc.vector.tensor_add(
                            h1_sb[:, tb, :], h1_sb[:, tb, :], l1b_sb
                        )

                # transpose LN1-normalized hidden -> feature-major for FFN
                tsrc = h1_sb if trivial_ln1 else h1n_sb
                for hb in range(HB):
                    pt = psm.tile([P, S], F32, tag="m", name=f"pt_{hb}")
                    for tb in range(TB):
                        nc.tensor.transpose(
                            pt[:, tb * P : (tb + 1) * P],
                            tsrc[:, tb, hb * P : (hb + 1) * P],
                            idf_sb,
                        )
                    if hb % 2 == 0:
                        nc.vector.tensor_copy(h1T_sb[:, hb, :], pt)
                    else:
                        nc.scalar.copy(h1T_sb[:, hb, :], pt)

            # ================= FFN scope =================
            with (
                tc.tile_pool(name="gpool", bufs=FB) as gpool,
                tc.tile_pool(name="ypool", bufs=1) as ypool,
            ):
                y_sb = ypool.tile([P, TB, H], F32, name="y_sb")
                g_tiles = []
                for f in range(FB):
                    w1t = wl_pool.tile([P, HB, P], F16, tag="wl", name=f"w1_{f}")
                    nc.sync.dma_start(
                        w1t,
                        d_w1.ap()[:, f * P : (f + 1) * P].rearrange(
                            "(ko p) m -> p ko m", p=P
                        ),
                    )
                    pf = psm.tile([P, S], F32, tag="m", name=f"pf_{f}")
                    for kb in range(HB):
                        nc.tensor.matmul(
                            pf, w1t[:, kb, :], h1T_sb[:, kb, :],
                            start=(kb == 0), stop=(kb == HB - 1),
                        )
                    g = gpool.tile([P, S], F16, tag="g", name=f"g_{f}")
                    nc.scalar.activation(
                        g, pf, AFT.Gelu, bias=b1_sb[:, f : f + 1]
                    )
                    g_tiles.append(g)

                for hf in range(2):
                    py = [
                        psh.tile([P, 384], F32, tag="h", name=f"py_{hf}_{tb}")
                        for tb in range(TB)
                    ]
                    for f in range(FB):
                        w2t = wr_pool.tile(
                            [P, 384], F16, tag="w2", name=f"w2_{hf}_{f}"
                        )
                        nc.sync.dma_start(
                            w2t,
                            d_w2.ap()[
                                f * P : (f + 1) * P, hf * 384 : (hf + 1) * 384
                            ],
                        )
                        for tb in range(TB):
                            nc.tensor.matmul(
                                py[tb],
                                g_tiles[f][:, tb * P : (tb + 1) * P],
                                w2t,
                                start=(f == 0), stop=False,
                            )
                    for tb in range(TB):
                        nc.tensor.matmul(
                            py[tb], onesr_sb, b2_sb[:, hf * 384 : (hf + 1) * 384],
                            start=False, stop=True,
                        )
                        nc.vector.tensor_add(
                            y_sb[:, tb, hf * 384 : (hf + 1) * 384],
                            py[tb],
                            h1_sb[:, tb, hf * 384 : (hf + 1) * 384],
                        )

                # LN2 -> output
                for tb in range(TB):
                    st = statp.tile([P, 2, 6], F32, tag="st", name=f"st2_{tb}")
                    nc.vector.bn_stats(st[:, 0, :], y_sb[:, tb, 0:384])
                    nc.vector.bn_stats(st[:, 1, :], y_sb[:, tb, 384:768])
                    ag = statp.tile([P, 2], F32, tag="ag", name=f"ag2_{tb}")
                    nc.vector.bn_aggr(ag, st)
                    sq = statp.tile([P, 1], F32, tag="sq", name=f"sq2_{tb}")
                    nc.scalar.activation(sq, ag[:, 1:2], AFT.Sqrt, bias=eps_sb)
                    rstd = statp.tile([P, 1], F32, tag="rstd", name=f"rstd2_{tb}")
                    nc.vector.reciprocal(rstd, sq)
                    o_sb = evp.tile([P, H], F32, tag="o", name=f"o_{tb}")
                    nc.vector.tensor_scalar(
                        o_sb, y_sb[:, tb, :], ag[:, 0:1], rstd,
                        ALU.subtract, ALU.mult,
                    )
                    if not trivial_ln2:
                        nc.vector.tensor_mul(o_sb, o_sb, l2s_sb)
                        nc.vector.tensor_add(o_sb, o_sb, l2b_sb)
                    nc.sync.dma_start(
                        d_out.ap()[tb * P : (tb + 1) * P, :], o_sb
                    )

    nc.compile()
    return nc


_CACHE = {}


def _get_nc(trivial_ln1, trivial_ln2):
    key = (trivial_ln1, trivial_ln2)
    if key not in _CACHE:
        _CACHE[key] = build(trivial_ln1, trivial_ln2)
    return _CACHE[key]


def _prepare(inputs):
    f32 = np.float32
    f16 = np.float16
    x = np.asarray(inputs["hidden_states"], f32)            # [B, S, H]
    mask = np.asarray(inputs["attention_mask"])
    assert mask.all(), "kernel assumes an all-true attention mask"
    Wq = np.asarray(inputs["Wq"], f32)
    bq = np.asarray(inputs["bq"], f32)
    Wk = np.asarray(inputs["Wk"], f32)
    bk = np.asarray(inputs["bk"], f32)
    Wv = np.asarray(inputs["Wv"], f32)
    bv = np.asarray(inputs["bv"], f32)
    Wo = np.asarray(inputs["Wo"], f32)
    bo = np.asarray(inputs["bo"], f32)
    rel = np.asarray(inputs["rel_table"], f32)              # [1023, 64]
    l1s = np.asarray(inputs["ln1_scale"], f32)
    l1b = np.asarray(inputs["ln1_bias"], f32)
    W1 = np.asarray(inputs["W1"], f32)
    b1 = np.asarray(inputs["b1"], f32)
    W2 = np.asarray(inputs["W2"], f32)
    b2 = np.asarray(inputs["b2"], f32)
    l2s = np.asarray(inputs["ln2_scale"], f32)
    l2b = np.asarray(inputs["ln2_bias"], f32)

    B = x.shape[0]
    trivial_ln1 = bool(np.all(l1s == 1.0) and np.all(l1b == 0.0))
    trivial_ln2 = bool(np.all(l2s == 1.0) and np.all(l2b == 0.0))

    # host-side folds (exact algebra)
    bo_p = bo + bv @ Wo                      # V-bias folded via softmax row-sum
    RT = np.zeros((HD, 1024), f16)
    RT[:, :1023] = (8.0 * rel[::-1].T).astype(f16)   # Q pre-scaled by 1/8
    W1f = l1s[:, None] * W1
    b1f = b1 + l1b @ W1

    common = {
        "wq": Wq.astype(f16),
        "wk": Wk.astype(f16),
        "wv": Wv.astype(f16),
        "wo": Wo.astype(f16),
        "w1": W1f.astype(f16),
        "w2": W2.astype(f16),
        "rt": RT,
        "bq8": np.ascontiguousarray(bq / 8.0),
        "bk": np.ascontiguousarray(bk),
        "b1f": np.ascontiguousarray(b1f),
        "bo_row": bo_p[None, :].astype(f16),
        "b2_row": b2[None, :].astype(f16),
        "ones_row": np.ones((1, P), f16),
        "ident_f16": np.eye(P, dtype=f16),
        "ident_f32": np.eye(P, dtype=f32),
    }
    if not trivial_ln1:
        common["ln1s_b"] = np.broadcast_to(l1s, (P, H)).copy()
        common["ln1b_b"] = np.broadcast_to(l1b, (P, H)).copy()
    if not trivial_ln2:
        common["ln2s_b"] = np.broadcast_to(l2s, (P, H)).copy()
        common["ln2b_b"] = np.broadcast_to(l2b, (P, H)).copy()

    in_maps = []
    for b in range(B):
        m = dict(common)
        m["xT"] = x[b].T.astype(f16)
        m["x_res"] = np.ascontiguousarray(x[b])
        in_maps.append(m)
    return in_maps, trivial_ln1, trivial_ln2, x.dtype


def run(inputs, trace=False, **kw):
    in_maps, t1, t2, dt = _prepare(inputs)
    nc = _get_nc(t1, t2)
    res = run_bass_kernel_spmd(
        nc, in_maps, core_ids=list(range(len(in_maps))), trace=trace, **kw
    )
    out = np.stack([res.results[c]["out"] for c in range(len(in_maps))])
    return out.astype(dt, copy=False), res


def kernel(**inputs) -> np.ndarray:
    out, _ = run(inputs, trace=False)
    return out
